# revision 19
# baseline (speedup 1.0000x reference)
"""Trainium2 Bass kernel for a 4-layer GNN (relational conv + LN + ELU + residual)
with boundary head and domain-adversarial head, distributed over 8 NeuronCores.

Strategy (graph partitioning, pure data parallel):
  - Nodes are partitioned into 8 contiguous ranges of 6250; core k owns range k
    and all edges whose DST falls in its range (segment-sum locality).
  - Within each core, own nodes are sorted by in-degree (descending) and laid
    out partition-minor: pi-position r -> (partition r%128, block r//128) in a
    [128, 49*128] SBUF accumulator. The k-th incoming edge of every node in a
    block forms one 128-row indirect-DMA gather call (one index per partition,
    which is what the runtime's dynamic-DMA descriptor generator supports);
    gathered tiles are segment-summed with VectorE adds. Degree sorting makes
    calls-per-block ~= avg degree of the block, minimizing call count.
  - Message GEMMs use linearity: segment_sum(x[src] @ W) == segment_sum(x[src]) @ W,
    so the per-edge GEMM collapses to one [6272,128]x[128,128] GEMM per layer.
  - The full x table (sigma-ordered, zero-padded) is replicated in DRAM; after
    each layer the updated shards are exchanged with an AllGather collective
    (~3.2MB/core), which runs on TOPSP+SDMA silicon and not the compute engines.
  - Small weights are replicated. Boundary logits and graph-pool partials are
    computed on device; the tiny [64,128] domain head runs on host.
"""
import os
import numpy as np

import concourse.bass as bass
import concourse.bacc as bacc
import concourse.tile as tile
from concourse import mybir, bass_utils
from concourse.masks import make_identity

N_NODES = 50000
N_EDGES = 600000
D = 128
L = 4
N_GRAPHS = 64
N_PATIENTS = 50
NCORES = 8
NPC = N_NODES // NCORES            # 6250 real nodes per core
NB = (NPC + 127) // 128            # 49 blocks
NOWN = NB * 128                    # 6272 padded node slots per core
NPAD = 16                          # zero rows appended to each core's shard
NSTRIDE = NOWN + NPAD              # 6288: table stride per core
TBL = NCORES * NSTRIDE             # 50304 table rows

_last_results = None               # test.py reads exec_time_ns from here
_build_cache = {}


def _bcast_row(ap_row, p=128):
    """Broadcast a [1, F] DRAM AP across p partitions (stride-0 partition dim)."""
    assert ap_row.shape[0] == 1
    return bass.AP(
        tensor=ap_row.tensor, offset=ap_row.offset, ap=[[0, p]] + list(ap_row.ap[1:])
    )


def build_nc(gmax):
    """Build the SPMD program. gmax[b] = static number of gather calls for
    block b (max over cores of the max in-degree within that block)."""
    ncalls = int(sum(gmax))
    f32 = mybir.dt.float32

    nc = bacc.Bacc("TRN2", target_bir_lowering=False, debug=False, num_devices=NCORES)

    x_tbl = nc.dram_tensor("x_tbl", [TBL, D], f32, kind="ExternalInput")
    x_own = nc.dram_tensor("x_own", [128, NOWN], f32, kind="ExternalInput")
    idx_all = nc.dram_tensor("idx_all", [128, ncalls], mybir.dt.int32, kind="ExternalInput")
    oneh = nc.dram_tensor("oneh", [128, NB * N_GRAPHS], f32, kind="ExternalInput")
    w_root = nc.dram_tensor("w_root", [L * D, D], f32, kind="ExternalInput")
    w_rel = nc.dram_tensor("w_rel", [L * D, D], f32, kind="ExternalInput")
    b_conv = nc.dram_tensor("b_conv", [L, D], f32, kind="ExternalInput")
    ln_g = nc.dram_tensor("ln_g", [L, D], f32, kind="ExternalInput")
    ln_b = nc.dram_tensor("ln_b", [L, D], f32, kind="ExternalInput")
    bw = nc.dram_tensor("bw", [1, D], f32, kind="ExternalInput")

    bnd_out = nc.dram_tensor("bnd", [128, NB], f32, kind="ExternalOutput")
    pool_out = nc.dram_tensor("pool", [N_GRAPHS, D], f32, kind="ExternalOutput")
    debug = bool(os.environ.get("KERNEL_DEBUG_L0"))
    if debug:
        dbg_acc = nc.dram_tensor("dbg_acc", [128, NOWN], f32, kind="ExternalOutput")
        dbg_x1 = nc.dram_tensor("dbg_x1", [128, NOWN], f32, kind="ExternalOutput")
        dbg_tbl = nc.dram_tensor("dbg_tbl", [256, D], f32, kind="ExternalOutput")

    with tile.TileContext(nc) as tc:
        with (
            tc.tile_pool(name="singles", bufs=1) as singles,
            tc.tile_pool(name="lconst", bufs=2) as lconst,
            tc.tile_pool(name="gpool", bufs=24) as gpool,
            tc.tile_pool(name="work", bufs=4) as work,
            tc.tile_pool(name="psum", bufs=2, space="PSUM") as psum,
            tc.tile_pool(name="psum1", bufs=1, space="PSUM") as psum1,
            tc.tile_pool(name="dram", bufs=1, space="DRAM") as dram,
        ):
            ident = singles.tile([128, 128], f32)
            make_identity(nc, ident[:])
            idx_sb = singles.tile([128, ncalls], mybir.dt.int32)
            nc.sync.dma_start(out=idx_sb[:], in_=idx_all[:])
            oneh_sb = singles.tile([128, NB * N_GRAPHS], f32)
            nc.sync.dma_start(out=oneh_sb[:], in_=oneh[:])
            bw_sb = singles.tile([128, D], f32)
            nc.sync.dma_start(out=bw_sb[:], in_=_bcast_row(bw[0:1, :]))

            # x state double buffer (partition-minor layout [128, NB*128])
            xa = singles.tile([128, NOWN], f32, tag="xa")
            xb = singles.tile([128, NOWN], f32, tag="xb")
            nc.sync.dma_start(out=xa[:], in_=x_own[:])
            x_cur, x_nxt = xa, xb

            acc = singles.tile([128, NOWN], f32, tag="acc")
            zero_sb = singles.tile([128, D], f32)
            nc.vector.memset(zero_sb[:], 0.0)
            eps_sb = singles.tile([128, 1], f32)
            nc.vector.memset(eps_sb[:], 1e-5)

            # AllGather staging: per-layer in/out DRAM tiles. Each shard is
            # padded with NPAD zero rows so the AG output covers the whole
            # table (zero rows included) with a single writer.
            ag_in = []
            ag_out = []
            for l in range(L - 1):
                t_in = dram.tile([NSTRIDE, D], f32, tag=f"agin{l}", name=f"agin{l}")
                nc.sync.dma_start(
                    out=t_in[NOWN:NSTRIDE, :], in_=zero_sb[0:NPAD, :]
                )
                ag_in.append(t_in)
                t = dram.tile(
                    [TBL, D], f32, tag=f"agout{l}", addr_space="Shared",
                    name=f"agout{l}",
                )
                ag_out.append(t)

            tables = [x_tbl[:]] + [t[:] for t in ag_out]

            for l in range(L):
                # ---- layer constants ----
                wroot_sb = lconst.tile([128, D], f32, tag="wroot")
                nc.sync.dma_start(out=wroot_sb[:], in_=w_root[l * D:(l + 1) * D, :])
                wrel_sb = lconst.tile([128, D], f32, tag="wrel")
                nc.sync.dma_start(out=wrel_sb[:], in_=w_rel[l * D:(l + 1) * D, :])
                bconv_sb = lconst.tile([128, D], f32, tag="bconv")
                nc.sync.dma_start(out=bconv_sb[:], in_=_bcast_row(b_conv[l:l + 1, :]))
                gam_sb = lconst.tile([128, D], f32, tag="gam")
                nc.sync.dma_start(out=gam_sb[:], in_=_bcast_row(ln_g[l:l + 1, :]))
                bet_sb = lconst.tile([128, D], f32, tag="bet")
                nc.sync.dma_start(out=bet_sb[:], in_=_bcast_row(ln_b[l:l + 1, :]))

                # ---- gather + segment-sum into acc ----
                c = 0
                tbl_ap = tables[l]
                for b in range(NB):
                    a_b = acc[:, b * 128:(b + 1) * 128]
                    g_first = None
                    for t in range(gmax[b]):
                        g = gpool.tile([128, D], f32, tag="g")
                        nc.gpsimd.indirect_dma_start(
                            out=g[:], out_offset=None, in_=tbl_ap,
                            in_offset=bass.IndirectOffsetOnAxis(
                                ap=idx_sb[:, c:c + 1], axis=0
                            ),
                        )
                        c += 1
                        if t == 0:
                            g_first = g
                        elif t == 1:
                            nc.vector.tensor_add(out=a_b, in0=g_first[:], in1=g[:])
                        else:
                            nc.vector.tensor_add(out=a_b, in0=a_b, in1=g[:])
                    if gmax[b] == 0:
                        nc.vector.memset(a_b, 0.0)
                    elif gmax[b] == 1:
                        nc.vector.tensor_copy(out=a_b, in_=g_first[:])
                assert c == ncalls if l == L - 1 else True

                # ---- per-chunk GEMM + LN + ELU + residual ----
                for b in range(NB):
                    x_c = x_cur[:, b * 128:(b + 1) * 128]
                    a_c = acc[:, b * 128:(b + 1) * 128]
                    xT_ps = psum.tile([128, 128], f32, tag="xT")
                    nc.tensor.transpose(out=xT_ps[:], in_=x_c, identity=ident[:])
                    xT_sb = work.tile([128, 128], f32, tag="xT_sb")
                    nc.vector.tensor_copy(out=xT_sb[:], in_=xT_ps[:])
                    aT_ps = psum.tile([128, 128], f32, tag="aT")
                    nc.tensor.transpose(out=aT_ps[:], in_=a_c, identity=ident[:])
                    aT_sb = work.tile([128, 128], f32, tag="aT_sb")
                    nc.vector.tensor_copy(out=aT_sb[:], in_=aT_ps[:])

                    h_ps = psum.tile([128, 128], f32, tag="h")
                    nc.tensor.matmul(
                        out=h_ps[:], lhsT=xT_sb[:], rhs=wroot_sb[:],
                        start=True, stop=False,
                    )
                    nc.tensor.matmul(
                        out=h_ps[:], lhsT=aT_sb[:], rhs=wrel_sb[:],
                        start=False, stop=True,
                    )

                    h0 = work.tile([128, 128], f32, tag="h0")
                    nc.vector.tensor_add(out=h0[:], in0=h_ps[:], in1=bconv_sb[:])
                    stats = work.tile([128, 6], f32, tag="stats")
                    nc.vector.bn_stats(out=stats[:], in_=h0[:])
                    mv = work.tile([128, 2], f32, tag="mv")
                    nc.vector.bn_aggr(out=mv[:], in_=stats[:])
                    # rsig = 1/sqrt(var + eps)
                    sig = work.tile([128, 1], f32, tag="sig")
                    nc.scalar.activation(
                        out=sig[:], in_=mv[:, 1:2],
                        func=mybir.ActivationFunctionType.Sqrt,
                        bias=eps_sb[:], scale=1.0,
                    )
                    rsig = work.tile([128, 1], f32, tag="rsig")
                    nc.vector.reciprocal(out=rsig[:], in_=sig[:])
                    # hn = (h0 - mu) * rsig
                    hn = work.tile([128, 128], f32, tag="hn")
                    nc.vector.tensor_scalar(
                        out=hn[:], in0=h0[:], scalar1=mv[:, 0:1], scalar2=rsig[:],
                        op0=mybir.AluOpType.subtract, op1=mybir.AluOpType.mult,
                    )
                    # hg = hn * gamma + beta
                    nc.vector.tensor_mul(out=hn[:], in0=hn[:], in1=gam_sb[:])
                    nc.vector.tensor_add(out=hn[:], in0=hn[:], in1=bet_sb[:])
                    # ELU: relu(x) + exp(min(x,0)) - 1
                    emin = work.tile([128, 128], f32, tag="emin")
                    nc.vector.tensor_scalar_min(out=emin[:], in0=hn[:], scalar1=0.0)
                    nc.scalar.activation(
                        out=emin[:], in_=emin[:],
                        func=mybir.ActivationFunctionType.Exp,
                        bias=0.0, scale=1.0,
                    )
                    nc.vector.tensor_scalar_max(out=hn[:], in0=hn[:], scalar1=0.0)
                    nc.vector.tensor_add(out=hn[:], in0=hn[:], in1=emin[:])
                    # x_next = (elu - 1) + x_in  ->  add then -1
                    nc.vector.tensor_add(out=hn[:], in0=hn[:], in1=x_c)
                    nc.vector.tensor_scalar_add(
                        out=x_nxt[:, b * 128:(b + 1) * 128], in0=hn[:], scalar1=-1.0
                    )
                    if l < L - 1:
                        # export this block to the AllGather input as it lands
                        nc.sync.dma_start(
                            out=ag_in[l][b * 128:(b + 1) * 128, :],
                            in_=x_nxt[:, b * 128:(b + 1) * 128],
                        )

                x_cur, x_nxt = x_nxt, x_cur

                if debug and l == int(os.environ["KERNEL_DEBUG_L0"]) - 1:
                    nc.sync.dma_start(out=dbg_acc[:], in_=acc[:])
                    nc.sync.dma_start(out=dbg_x1[:], in_=x_cur[:])
                if debug and l == 1:
                    tb = work.tile([128, 2 * D], f32, tag="tb")
                    nc.sync.dma_start(
                        out=tb[:, 0:D],
                        in_=tables[1][0:128, :],
                    )
                    nc.sync.dma_start(
                        out=tb[:, D:2 * D],
                        in_=tables[1][NSTRIDE:NSTRIDE + 128, :],
                    )
                    nc.sync.dma_start(
                        out=dbg_tbl[0:128, :], in_=tb[:, 0:D]
                    )
                    nc.sync.dma_start(
                        out=dbg_tbl[128:256, :], in_=tb[:, D:2 * D]
                    )

                # ---- exchange updated shards (not needed after last layer) ----
                if l < L - 1:
                    nc.gpsimd.collective_compute(
                        "AllGather",
                        mybir.AluOpType.bypass,
                        replica_groups=[list(range(NCORES))],
                        ins=[ag_in[l][:]],
                        outs=[ag_out[l][:]],
                    )

            # ---- boundary logits: reduce(x * bw) per chunk ----
            bnd_sb = singles.tile([128, NB], f32)
            for b in range(NB):
                m = work.tile([128, 128], f32, tag="m")
                nc.vector.tensor_mul(
                    out=m[:], in0=x_cur[:, b * 128:(b + 1) * 128], in1=bw_sb[:]
                )
                nc.vector.tensor_reduce(
                    out=bnd_sb[:, b:b + 1], in_=m[:],
                    axis=mybir.AxisListType.X, op=mybir.AluOpType.add,
                )
            nc.sync.dma_start(out=bnd_out[:], in_=bnd_sb[:])

            # ---- graph pooling partials: onehot^T @ x accumulated on PE ----
            pool_ps = psum1.tile([N_GRAPHS, D], f32)
            for b in range(NB):
                nc.tensor.matmul(
                    out=pool_ps[:],
                    lhsT=oneh_sb[:, b * N_GRAPHS:(b + 1) * N_GRAPHS],
                    rhs=x_cur[:, b * 128:(b + 1) * 128],
                    start=(b == 0), stop=(b == NB - 1),
                )
            pool_sb = singles.tile([N_GRAPHS, D], f32)
            nc.vector.tensor_copy(out=pool_sb[:], in_=pool_ps[:])
            nc.sync.dma_start(out=pool_out[:], in_=pool_sb[:])

    nc.compile()
    return nc


def preprocess(x, edge_index, batch):
    """Host-side graph partitioning: per-core degree-sorted node order (sigma),
    gather-call index columns, one-hot pooling matrices."""
    src = edge_index[0].astype(np.int64)
    dst = edge_index[1].astype(np.int64)

    # per-core pi order (degree descending within each core's node range)
    deg = np.bincount(dst, minlength=N_NODES)
    pos_of = np.empty(N_NODES, dtype=np.int64)    # orig node -> pi position in core
    order_of = []                                  # core -> orig node ids by position
    for k in range(NCORES):
        lo = k * NPC
        d_k = deg[lo:lo + NPC]
        order = np.argsort(-d_k, kind="stable")    # local ids by degree desc
        order_of.append(order + lo)
        pos_of[order + lo] = np.arange(NPC)

    sigma = (np.arange(N_NODES) // NPC) * NSTRIDE + pos_of  # orig node -> table row

    # per-core sorted edge lists (by destination pi position)
    core_of_dst = dst // NPC
    dpos = pos_of[dst]                                    # dest pi position
    gmax = np.zeros(NB, dtype=np.int64)
    per_core = []
    for k in range(NCORES):
        m = core_of_dst == k
        s_k = src[m]
        p_k = dpos[m]
        order = np.argsort(p_k, kind="stable")
        s_k, p_k = s_k[order], p_k[order]
        # edge slot within node: running count per position
        starts = np.searchsorted(p_k, np.arange(NPC), side="left")
        ends = np.searchsorted(p_k, np.arange(NPC), side="right")
        d_k = ends - starts
        # gmax per block for this core (degree desc -> first node of block)
        for b in range(NB):
            lo = b * 128
            if lo < NPC:
                gmax[b] = max(gmax[b], d_k[lo])
        per_core.append((s_k, starts, d_k))

    ncalls = int(gmax.sum())
    idx_cols = []
    oneh_all = []
    for k in range(NCORES):
        s_k, starts, d_k = per_core[k]
        idx = np.empty((128, ncalls), dtype=np.int32)
        c = 0
        p128 = np.arange(128)
        zrow = ((p128 % 8) * NSTRIDE + NOWN + p128 // 8).astype(np.int32)
        for b in range(NB):
            base = b * 128
            for t in range(int(gmax[b])):
                col = zrow.copy()
                pos = base + np.arange(128)
                valid = pos < NPC
                posv = pos[valid]
                has = t < d_k[posv]
                rows = np.where(valid)[0][has]
                col[rows] = sigma[s_k[starts[posv[has]] + t]].astype(np.int32)
                idx[:, c] = col
                c += 1
        idx_cols.append(idx)
        # one-hot pooling matrix in pi order, partition-minor
        oh = np.zeros((128, NB * N_GRAPHS), dtype=np.float32)
        orig = order_of[k]                  # position -> orig node id
        g_of_pos = batch[orig]              # [NPC]
        pos = np.arange(NPC)
        oh[pos % 128, (pos // 128) * N_GRAPHS + g_of_pos] = 1.0
        oneh_all.append(oh)

    # sigma-ordered padded x table
    x_tbl = np.zeros((TBL, D), dtype=np.float32)
    x_tbl[sigma] = x
    return sigma, order_of, gmax, idx_cols, oneh_all, x_tbl


def kernel(x, edge_index, batch, W_root, W_rel, b_conv, ln_gamma, ln_beta,
           boundary_W, boundary_b, dom_W1, dom_b1, dom_W2, dom_b2):
    global _last_results
    x = np.asarray(x, dtype=np.float32)
    edge_index = np.asarray(edge_index, dtype=np.int32)
    batch = np.asarray(batch, dtype=np.int32)
    W_root = np.asarray(W_root, dtype=np.float32)
    W_rel = np.asarray(W_rel, dtype=np.float32)
    b_conv = np.asarray(b_conv, dtype=np.float32)
    ln_gamma = np.asarray(ln_gamma, dtype=np.float32)
    ln_beta = np.asarray(ln_beta, dtype=np.float32)
    boundary_W = np.asarray(boundary_W, dtype=np.float32)
    boundary_b = np.asarray(boundary_b, dtype=np.float32)
    dom_W1 = np.asarray(dom_W1, dtype=np.float32)
    dom_b1 = np.asarray(dom_b1, dtype=np.float32)
    dom_W2 = np.asarray(dom_W2, dtype=np.float32)
    dom_b2 = np.asarray(dom_b2, dtype=np.float32)

    sigma, order_of, gmax, idx_cols, oneh_all, x_tbl = preprocess(
        x, edge_index, batch
    )

    key = tuple(gmax.tolist())
    if key not in _build_cache:
        _build_cache[key] = build_nc(gmax)
    nc = _build_cache[key]

    wr = W_root.reshape(L * D, D)
    wl = W_rel.reshape(L * D, D)
    in_maps = []
    for k in range(NCORES):
        xo = x_tbl[k * NSTRIDE:k * NSTRIDE + NOWN].reshape(NB, 128, D)
        xo = xo.transpose(1, 0, 2).reshape(128, NOWN)  # partition-minor
        in_maps.append({
            "x_tbl": x_tbl,
            "x_own": np.ascontiguousarray(xo),
            "idx_all": idx_cols[k],
            "oneh": oneh_all[k],
            "w_root": wr, "w_rel": wl,
            "b_conv": b_conv, "ln_g": ln_gamma, "ln_b": ln_beta,
            "bw": boundary_W.reshape(1, D),
        })

    res = bass_utils.run_bass_kernel_spmd(nc, in_maps, core_ids=list(range(NCORES)))
    _last_results = res

    boundary = np.empty(N_NODES, dtype=np.float32)
    pooled = np.zeros((N_GRAPHS, D), dtype=np.float32)
    for k in range(NCORES):
        r = res.results[k]
        bnd = r["bnd"]                      # [128, NB]
        pos = np.arange(NPC)
        boundary[order_of[k]] = bnd[pos % 128, pos // 128]
        pooled += r["pool"]
    boundary += boundary_b[0]

    counts = np.bincount(batch, minlength=N_GRAPHS).astype(np.float32)
    pooled = pooled / np.clip(counts, 1.0, None)[:, None]
    h = pooled @ dom_W1 + dom_b1
    h = np.where(h > 0, h, np.expm1(h))     # ELU
    domain = h @ dom_W2 + dom_b2
    return boundary, domain
